# revision 17
# baseline (speedup 1.0000x reference)
"""Trainium2 Bass kernel for nn_LinearLatentKernel_84834194031187.

Computes, for x:[B,S,D], W_qkv:[3D,D], W_gate:[D,D] (fp32):
    qkv = x @ W_qkv.T + b_qkv ; q,k,v = split(qkv)
    kv_state = cumsum(k*v, axis=seq)
    out = q * kv_state * sigmoid(x @ W_gate.T + b_gate)

Sharding: 8 cores = (batch b in 0..3) x (channel half h in 0..1). Each core
handles x[b] [S,D] against a host-pretransposed weight slice W^T [D, 4*H]
(k,v,q,gate halves of H=512 channels each) and produces out[b,:,h*H:(h+1)*H].

x is host-pretransposed and pre-tiled into [NBLK, 128, KT, 128] fp16 so each
seq block's x^T tiles arrive via one contiguous DMA (2KB/partition) and feed
the PE stationary port directly -- no on-device transposes.

Per seq block of 128 rows (partition dim = seq):
  - k/v chunks [128, 512] accumulate in PSUM over 8 contraction tiles
    (fp16 operands, fp32 PSUM), then q/g chunks; this ordering lets the
    k/v banks be drained (kv = k*v on DVE) while q/g matmuls still run,
    so all four PSUM accumulators are single-buffered without stalls.
  - carry fold: kv[0,:] += carry (DVE add reading the carry PSUM bank
    directly); then one upper-triangular matmul gives the block cumsum
    INCLUDING the carry, and one ones-column matmul over the folded kv
    gives the NEXT carry (colsum(kv + e0 x carry) = carry + colsum(kv)).
    All cumsum matmuls run with fp16 operands.
  - The PE queue is software-pipelined one block: cumsum matmuls for
    block i-1 are enqueued after the projection matmuls of block i, so
    the PE never waits on the DVE's kv product.
  - out = (q * sigmoid(g)) * kv_state, streamed back to DRAM.
"""

import numpy as np

import concourse.bass as bass
import concourse.bacc as bacc
import concourse.tile as tile
import concourse.mybir as mybir
from concourse.bass_utils import run_bass_kernel_spmd

B, S, D = 4, 4096, 1024
H = 512          # channels per core (half of D)
P = 128
NBLK = S // P    # 32 seq blocks
KT = D // P      # 8 contraction tiles

f32 = mybir.dt.float32
f16 = mybir.dt.float16

_NC_CACHE = {}


def _build(with_bias: bool):
    nc = bacc.Bacc("TRN2", target_bir_lowering=False)

    # x^T pre-tiled on host: xh[i, p, kt, j] = x[i*128+j, kt*128+p]
    xh_d = nc.dram_tensor("xh", [NBLK, P, KT, P], f16, kind="ExternalInput")
    # weight columns ordered [k | v | q | g], H channels each
    wt_d = nc.dram_tensor("wt", [D, 4 * H], f16, kind="ExternalInput")
    tri_d = nc.dram_tensor("tri", [P, P], f16, kind="ExternalInput")
    if with_bias:
        onesrow_d = nc.dram_tensor("onesrow", [1, P], f16, kind="ExternalInput")
        bias_d = nc.dram_tensor("bias", [1, 4 * H], f16, kind="ExternalInput")
    out_d = nc.dram_tensor("out", [S, H], f32, kind="ExternalOutput")

    with tile.TileContext(nc) as tc:
        with (
            tc.tile_pool(name="consts", bufs=1) as consts,
            tc.tile_pool(name="xtp", bufs=4) as xtp,
            tc.tile_pool(name="kp", bufs=2) as kp,
            tc.tile_pool(name="gp", bufs=2) as gp,
            tc.tile_pool(name="kvp", bufs=2) as kvp,
            tc.tile_pool(name="qgp", bufs=2) as qgp,
            tc.tile_pool(name="outp", bufs=3) as outp,
            tc.tile_pool(name="tmpp", bufs=2) as tmpp,
            tc.tile_pool(name="carryp", bufs=2) as carryp,
            tc.tile_pool(name="pmm", bufs=1, space="PSUM") as pmm,
            tc.tile_pool(name="pcs_pool", bufs=2, space="PSUM") as pcs_pool,
            tc.tile_pool(name="pwm", bufs=1, space="PSUM") as pwm,
        ):
            # PE warmup: dummy matmuls with no DMA dependencies keep the PE
            # busy through the initial DMA ramp and move HAM into its fast
            # (K=8/8) state before the first real projection matmul. More
            # warm matmuls are interleaved into block 0 (see proj) to pad
            # the PE at the weight-DMA arrival pace.
            warm_a = consts.tile([P, P], f16, tag="warm_a")
            nc.vector.memset(warm_a[:], 0.0)
            warm_b = consts.tile([P, 512], f16, tag="warm_b")
            nc.vector.memset(warm_b[:], 0.0)
            pwarm = pwm.tile([P, 512], f32, tag="pwarm")

            def warm(n):
                for _ in range(n):
                    nc.tensor.matmul(pwarm[:], warm_a[:], warm_b[:],
                                     start=True, stop=True)

            warm(26)
            # x blocks 0/1 first so projections can start before W^T lands
            xt0 = xtp.tile([P, KT, P], f16, tag="xt", name="xt0")
            nc.sync.dma_start(xt0[:], xh_d[0])
            xt1 = xtp.tile([P, KT, P], f16, tag="xt", name="xt1")
            nc.sync.dma_start(xt1[:], xh_d[1])

            # W^T split per contraction tile and per (kv, qg) half, spread
            # over three engine-trigger queues to parallelize the rampup.
            # k/v halves (needed first) alternate gpsimd/sync in kt order.
            wt_sb = consts.tile([P, KT, 4 * H], f16, tag="wt")
            for kt in range(KT):
                eng = nc.gpsimd if kt % 2 == 0 else nc.sync
                eng.dma_start(wt_sb[:, kt, 0:2 * H],
                              wt_d[kt * P:(kt + 1) * P, 0:2 * H])
            for kt in range(KT):
                eng = nc.scalar if kt % 2 == 0 else nc.gpsimd
                eng.dma_start(wt_sb[:, kt, 2 * H:4 * H],
                              wt_d[kt * P:(kt + 1) * P, 2 * H:4 * H])

            tri_sb = consts.tile([P, P], f16, tag="tri")
            nc.sync.dma_start(tri_sb[:], tri_d[:])
            if with_bias:
                onesrow_sb = consts.tile([1, P], f16, tag="onesrow")
                nc.sync.dma_start(onesrow_sb[:], onesrow_d[:])
                bias_sb = consts.tile([1, 4 * H], f16, tag="bias")
                nc.sync.dma_start(bias_sb[:], bias_d[:])

            xts = {0: xt0, 1: xt1}
            pca_prev = None     # [1,H] PSUM: carry entering the current block
            pending = None      # (kv, qg, i) awaiting cumsum+output

            def proj(ps, xt, c0, c1, pad=0):
                # accumulate channel groups c0,c1 over all contraction tiles;
                # pad>0 inserts warm matmuls per kt (block 0: matches the
                # PE's consumption rate to the weight-DMA arrival rate)
                for kt in range(KT):
                    for ci, c in enumerate((c0, c1)):
                        nc.tensor.matmul(
                            ps[ci][:], xt[:, kt, :],
                            wt_sb[:, kt, c * H:(c + 1) * H],
                            start=(kt == 0),
                            stop=(kt == KT - 1 and not with_bias),
                        )
                    warm(pad)
                if with_bias:
                    for ci, c in enumerate((c0, c1)):
                        nc.tensor.matmul(
                            ps[ci][:], onesrow_sb[:],
                            bias_sb[:, c * H:(c + 1) * H],
                            start=False, stop=True,
                        )

            def cumsum_mms(pend):
                # PE part of block j's cumsum (runs pipelined one block late).
                # Row 127 of the cumsum IS the carry for block j+1 (the kv fed
                # here already has the incoming carry folded into row 0); a
                # 1-lane scalar copy + SBUF-to-SBUF DMA moves it from
                # partition 127 to partition 0 for the next fold.
                kv, qg, j = pend
                pcs = pcs_pool.tile([P, H], f32, tag="pcs")
                nc.tensor.matmul(pcs[:], tri_sb[:], kv[:], start=True, stop=True)
                carry_new = None
                if j < NBLK - 1:
                    # engines need 32-aligned partition bases: copy rows
                    # 96..127, then DMA only row 127 down to partition 0
                    tmp = tmpp.tile([P, H], f32, tag="tmp")
                    nc.scalar.activation(tmp[96:P, :], pcs[96:P, :],
                                         mybir.ActivationFunctionType.Copy)
                    carry_new = carryp.tile([1, H], f32, tag="carry")
                    nc.gpsimd.dma_start(carry_new[0:1, :], tmp[P - 1:P, :])
                return pcs, carry_new

            def emit_out(pend, pcs):
                _, qg, j = pend
                ob = outp.tile([P, H], f32, tag="ob")
                nc.vector.tensor_mul(out=ob[:], in0=qg[:], in1=pcs[:])
                nc.sync.dma_start(out_d[j * P:(j + 1) * P, :], ob[:])

            for i in range(NBLK):
                if i + 2 < NBLK:
                    xt = xtp.tile([P, KT, P], f16, tag="xt")
                    nc.sync.dma_start(xt[:], xh_d[i + 2])
                    xts[i + 2] = xt
                xt = xts.pop(i)

                ps_k = pmm.tile([P, H], f32, tag="psk", name="psk")
                ps_v = pmm.tile([P, H], f32, tag="psv", name="psv")
                ps_q = pmm.tile([P, H], f32, tag="psq", name="psq")
                ps_g = pmm.tile([P, H], f32, tag="psg", name="psg")
                proj((ps_k, ps_v), xt, 0, 1,
                     pad=2 if i == 0 else 0)   # k, v first: drained early

                # block i-1's cumsum matmuls run between the two projection
                # phases; its kv' product was folded+ready a block ago
                pcs_prev = None
                if pending is not None:
                    pcs_prev, carry_prev = cumsum_mms(pending)

                proj((ps_q, ps_g), xt, 2, 3, pad=2 if i == 0 else 0)

                k_sb = kp.tile([P, H], f32, tag="k")
                nc.scalar.activation(k_sb[:], ps_k[:],
                                     mybir.ActivationFunctionType.Copy)
                g_sb = gp.tile([P, H], f32, tag="g")
                nc.scalar.activation(g_sb[:], ps_g[:],
                                     mybir.ActivationFunctionType.Sigmoid)
                kv = kvp.tile([P, H], f16, tag="kv")
                nc.vector.tensor_mul(out=kv[:], in0=k_sb[:], in1=ps_v[:])
                if pending is not None:
                    emit_out(pending, pcs_prev)
                qg = qgp.tile([P, H], f32, tag="qg")
                nc.vector.tensor_mul(out=qg[:], in0=g_sb[:], in1=ps_q[:])

                if i > 0:
                    # carry fold: kv[0,:] += carry (block i-1's cumsum row 127)
                    nc.vector.tensor_add(out=kv[0:1, :], in0=kv[0:1, :],
                                         in1=carry_prev[:])
                pending = (kv, qg, i)

            pcs_last, _ = cumsum_mms(pending)
            emit_out(pending, pcs_last)

    nc.compile()
    return nc


def _get_nc(with_bias: bool):
    if with_bias not in _NC_CACHE:
        _NC_CACHE[with_bias] = _build(with_bias)
    return _NC_CACHE[with_bias]


def _prep_in_maps(x, W_qkv, b_qkv, W_gate, b_gate, with_bias):
    x = np.asarray(x, dtype=np.float32).astype(np.float16)
    W_qkv = np.asarray(W_qkv, dtype=np.float32)
    W_gate = np.asarray(W_gate, dtype=np.float32)

    consts = {
        "tri": np.triu(np.ones((P, P), dtype=np.float16)),
    }
    if with_bias:
        consts["onesrow"] = np.ones((1, P), dtype=np.float16)

    # xh[b][i, p, kt, j] = x[b, i*128+j, kt*128+p]
    xhs = [
        np.ascontiguousarray(
            x[b].reshape(NBLK, P, KT, P).transpose(0, 3, 2, 1))
        for b in range(B)
    ]

    wts, biases = [], []
    for h in range(2):
        sl = slice(h * H, (h + 1) * H)
        wt = np.concatenate(
            [W_qkv[D + h * H:D + (h + 1) * H],        # k rows
             W_qkv[2 * D + h * H:2 * D + (h + 1) * H],  # v rows
             W_qkv[sl],                                 # q rows
             W_gate[sl]], axis=0                        # g rows
        ).T
        wts.append(np.ascontiguousarray(wt).astype(np.float16))
        if with_bias:
            bq = np.asarray(b_qkv, dtype=np.float32)
            bg = np.asarray(b_gate, dtype=np.float32)
            biases.append(np.concatenate(
                [bq[D + h * H:D + (h + 1) * H],
                 bq[2 * D + h * H:2 * D + (h + 1) * H],
                 bq[sl], bg[sl]]
            )[None, :].astype(np.float16).copy())

    in_maps = []
    for core in range(8):
        b, h = core // 2, core % 2
        m = {"xh": xhs[b], "wt": wts[h], **consts}
        if with_bias:
            m["bias"] = biases[h]
        in_maps.append(m)
    return in_maps


def run(x, W_qkv, b_qkv, W_gate, b_gate, trace=False, **run_kwargs):
    with_bias = bool(np.any(np.asarray(b_qkv)) or np.any(np.asarray(b_gate)))
    nc = _get_nc(with_bias)
    in_maps = _prep_in_maps(x, W_qkv, b_qkv, W_gate, b_gate, with_bias)
    res = run_bass_kernel_spmd(nc, in_maps, list(range(8)), trace=trace, **run_kwargs)
    out = np.empty((B, S, D), dtype=np.float32)
    for core in range(8):
        b, h = core // 2, core % 2
        out[b, :, h * H:(h + 1) * H] = res.results[core]["out"]
    return out, res


def kernel(x, W_qkv, b_qkv, W_gate, b_gate):
    out, _ = run(x, W_qkv, b_qkv, W_gate, b_gate)
    return out


# revision 20
# speedup vs baseline: 1.0046x; 1.0046x over previous
"""Trainium2 Bass kernel for nn_LinearLatentKernel_84834194031187.

Computes, for x:[B,S,D], W_qkv:[3D,D], W_gate:[D,D] (fp32):
    qkv = x @ W_qkv.T + b_qkv ; q,k,v = split(qkv)
    kv_state = cumsum(k*v, axis=seq)
    out = q * kv_state * sigmoid(x @ W_gate.T + b_gate)

Sharding: 8-way channel split. Core h handles channels [h*128,(h+1)*128) of
q, k, v and the gate for ALL four batches, producing out[:, :, h*128:...].
This keeps each core's weight slice at 1MB (vs 4MB for a batch x half-D
split), which matters because the startup ramp is HBM-bandwidth-bound:
block 0 cannot finish before its weights land.

x is host-pretransposed and pre-tiled into [NBLK, 128, B, KT, 128] fp16 so
each seq block's x^T tiles (all 4 batches) arrive via contiguous DMAs
(8KB/partition) and feed the PE stationary port directly -- no on-device
transposes.

Per seq block of 128 rows (partition dim = seq):
  - one PSUM bank per batch accumulates [k|v|q|g] x 128 channels over the
    8 contraction tiles (fp16 operands, fp32 PSUM, N=512 matmuls). The
    batch-outer matmul order staggers bank completion so each bank is
    drained (kv product, sigmoid, q*g on DVE/ACT) while later batches'
    matmuls still run: all four banks stay single-buffered with no stalls.
  - carry fold: kv_b[0,:] += carry_b; one upper-triangular fp16 matmul per
    batch then yields the block cumsum INCLUDING the carry, and its row 127
    IS the carry for the next block: a 1-lane scalar copy + SBUF-to-SBUF
    DMA (all 4 batches in one go) moves it from partition 127 to 0.
  - The PE queue is software-pipelined one block: cumsum matmuls for block
    i-1 are enqueued between batch groups of block i's projections.
  - out = (q * sigmoid(g)) * kv_state, one 256KB DMA per block.

A short burst of dummy "warmup" matmuls (no DMA dependencies) runs first so
the PE's HAM reaches its fast state during the initial DMA ramp.
"""

import numpy as np

import concourse.bacc as bacc
import concourse.tile as tile
import concourse.mybir as mybir
from concourse.bass_utils import run_bass_kernel_spmd

B, S, D = 4, 4096, 1024
HC = 128         # channels per core (D / 8 cores)
W4 = 4 * HC      # k|v|q|g channel block per core = 512
P = 128
NBLK = S // P    # 32 seq blocks
KT = D // P      # 8 contraction tiles

f32 = mybir.dt.float32
f16 = mybir.dt.float16

_NC_CACHE = {}


def _build(with_bias: bool):
    nc = bacc.Bacc("TRN2", target_bir_lowering=False)

    # x^T pre-tiled on host: xh[i, p, b, kt, j] = x[b, i*128+j, kt*128+p]
    xh_d = nc.dram_tensor("xh", [NBLK, P, B, KT, P], f16, kind="ExternalInput")
    # weight columns ordered [k | v | q | g], HC channels each
    wt_d = nc.dram_tensor("wt", [KT, P, W4], f16, kind="ExternalInput")
    tri_d = nc.dram_tensor("tri", [P, P], f16, kind="ExternalInput")
    if with_bias:
        onesrow_d = nc.dram_tensor("onesrow", [1, P], f16, kind="ExternalInput")
        bias_d = nc.dram_tensor("bias", [1, W4], f16, kind="ExternalInput")
    # out[i, j, b, c] = result[b, i*128+j, h*128+c]
    out_d = nc.dram_tensor("out", [NBLK, P, B, HC], f32, kind="ExternalOutput")

    with tile.TileContext(nc) as tc:
        with (
            tc.tile_pool(name="consts", bufs=1) as consts,
            tc.tile_pool(name="xtp", bufs=4) as xtp,
            tc.tile_pool(name="kp", bufs=2) as kp,
            tc.tile_pool(name="gp", bufs=2) as gp,
            tc.tile_pool(name="kvp", bufs=2) as kvp,
            tc.tile_pool(name="qgp", bufs=2) as qgp,
            tc.tile_pool(name="outp", bufs=3) as outp,
            tc.tile_pool(name="tmpp", bufs=2) as tmpp,
            tc.tile_pool(name="carryp", bufs=2) as carryp,
            tc.tile_pool(name="pmm", bufs=1, space="PSUM") as pmm,
            tc.tile_pool(name="pcs_pool", bufs=2, space="PSUM") as pcs_pool,
            tc.tile_pool(name="pwm", bufs=1, space="PSUM") as pwm,
        ):
            warm_a = consts.tile([P, P], f16, tag="warm_a")
            nc.vector.memset(warm_a[:], 0.0)
            warm_b = consts.tile([P, 512], f16, tag="warm_b")
            nc.vector.memset(warm_b[:], 0.0)
            pwarm = pwm.tile([P, 512], f32, tag="pwarm")

            def warm(n):
                for _ in range(n):
                    nc.tensor.matmul(pwarm[:], warm_a[:], warm_b[:],
                                     start=True, stop=True)

            warm(22)

            # block 0/1 inputs split across two trigger queues; weights on
            # gpsimd+scalar so everything rides different DMA rings
            xt0 = xtp.tile([P, B, KT, P], f16, tag="xt", name="xt0")
            nc.sync.dma_start(xt0[:, 0:2], xh_d[0, :, 0:2])
            nc.scalar.dma_start(xt0[:, 2:4], xh_d[0, :, 2:4])
            wt_sb = consts.tile([P, KT, W4], f16, tag="wt")
            for kt in range(KT):
                eng = nc.gpsimd if kt % 2 == 0 else nc.scalar
                eng.dma_start(wt_sb[:, kt, :], wt_d[kt])
            xt1 = xtp.tile([P, B, KT, P], f16, tag="xt", name="xt1")
            nc.sync.dma_start(xt1[:, 0:2], xh_d[1, :, 0:2])
            nc.scalar.dma_start(xt1[:, 2:4], xh_d[1, :, 2:4])

            tri_sb = consts.tile([P, P], f16, tag="tri")
            nc.sync.dma_start(tri_sb[:], tri_d[:])
            if with_bias:
                onesrow_sb = consts.tile([1, P], f16, tag="onesrow")
                nc.sync.dma_start(onesrow_sb[:], onesrow_d[:])
                bias_sb = consts.tile([1, W4], f16, tag="bias")
                nc.sync.dma_start(bias_sb[:], bias_d[:])

            xts = {0: xt0, 1: xt1}
            pending = None      # (kvs, qgs, i) awaiting cumsum+output

            def proj_batch(ps_b, xt, b):
                for kt in range(KT):
                    nc.tensor.matmul(
                        ps_b[:], xt[:, b, kt, :], wt_sb[:, kt, :],
                        start=(kt == 0),
                        stop=(kt == KT - 1 and not with_bias),
                    )
                if with_bias:
                    nc.tensor.matmul(ps_b[:], onesrow_sb[:], bias_sb[:],
                                     start=False, stop=True)

            def cumsum_mms(pend):
                # PE part of block j's cumsum, one tri matmul per batch into
                # quarter-bank slices; row 127 = next carry (all batches),
                # moved 127->0 via 1-lane copy + tiny SBUF DMA.
                kvs, qgs, j = pend
                pcs = pcs_pool.tile([P, B, HC], f32, tag="pcs")
                for b in range(B):
                    nc.tensor.matmul(pcs[:, b, :], tri_sb[:], kvs[b][:],
                                     start=True, stop=True)
                carry_new = None
                if j < NBLK - 1:
                    tmp = tmpp.tile([P, B, HC], f32, tag="tmp")
                    nc.scalar.activation(tmp[96:P], pcs[96:P],
                                         mybir.ActivationFunctionType.Copy)
                    carry_new = carryp.tile([1, B, HC], f32, tag="carry")
                    nc.gpsimd.dma_start(carry_new[0:1], tmp[P - 1:P])
                return pcs, carry_new

            def emit_out(pend, pcs):
                _, qgs, j = pend
                ob = outp.tile([P, B, HC], f32, tag="ob")
                for b in range(B):
                    nc.vector.tensor_mul(out=ob[:, b, :], in0=qgs[b][:],
                                         in1=pcs[:, b, :])
                nc.gpsimd.dma_start(out_d[j], ob[:])

            for i in range(NBLK):
                if i + 2 < NBLK:
                    xt = xtp.tile([P, B, KT, P], f16, tag="xt")
                    nc.sync.dma_start(xt[:, 0:2], xh_d[i + 2, :, 0:2])
                    nc.scalar.dma_start(xt[:, 2:4], xh_d[i + 2, :, 2:4])
                    xts[i + 2] = xt
                xt = xts.pop(i)

                ps = [pmm.tile([P, W4], f32, tag=f"ps{b}", name=f"ps{b}")
                      for b in range(B)]
                kvs, qgs = [], []

                proj_batch(ps[0], xt, 0)
                proj_batch(ps[1], xt, 1)

                # block i-1's cumsum matmuls, mid-block on the PE queue
                pcs_prev = None
                if pending is not None:
                    pcs_prev, carry_prev = cumsum_mms(pending)

                proj_batch(ps[2], xt, 2)
                proj_batch(ps[3], xt, 3)

                for b in range(B):
                    k_sb = kp.tile([P, HC], f32, tag=f"k{b}")
                    nc.scalar.activation(k_sb[:], ps[b][:, 0:HC],
                                         mybir.ActivationFunctionType.Copy)
                    kv = kvp.tile([P, HC], f16, tag=f"kv{b}")
                    nc.vector.tensor_mul(out=kv[:], in0=k_sb[:],
                                         in1=ps[b][:, HC:2 * HC])
                    kvs.append(kv)
                    if b == 0 and pending is not None:
                        emit_out(pending, pcs_prev)
                    g_sb = gp.tile([P, HC], f32, tag=f"g{b}")
                    nc.scalar.activation(g_sb[:], ps[b][:, 3 * HC:4 * HC],
                                         mybir.ActivationFunctionType.Sigmoid)
                    qg = qgp.tile([P, HC], f32, tag=f"qg{b}")
                    nc.vector.tensor_mul(out=qg[:], in0=g_sb[:],
                                         in1=ps[b][:, 2 * HC:3 * HC])
                    qgs.append(qg)

                if i > 0:
                    for b in range(B):
                        # carry fold: kv_b[0,:] += carry (cumsum row 127)
                        nc.vector.tensor_add(
                            out=kvs[b][0:1, :], in0=kvs[b][0:1, :],
                            in1=carry_prev[0:1, b, :])
                pending = (kvs, qgs, i)

            pcs_last, _ = cumsum_mms(pending)
            emit_out(pending, pcs_last)

    nc.compile()
    return nc


def _get_nc(with_bias: bool):
    if with_bias not in _NC_CACHE:
        _NC_CACHE[with_bias] = _build(with_bias)
    return _NC_CACHE[with_bias]


def _prep_in_maps(x, W_qkv, b_qkv, W_gate, b_gate, with_bias):
    x = np.asarray(x, dtype=np.float32).astype(np.float16)
    W_qkv = np.asarray(W_qkv, dtype=np.float32)
    W_gate = np.asarray(W_gate, dtype=np.float32)

    consts = {
        "tri": np.triu(np.ones((P, P), dtype=np.float16)),
    }
    if with_bias:
        consts["onesrow"] = np.ones((1, P), dtype=np.float16)

    # xh[i, p, b, kt, j] = x[b, i*128+j, kt*128+p]  (shared by all cores)
    xh = np.ascontiguousarray(
        x.reshape(B, NBLK, P, KT, P).transpose(1, 4, 0, 3, 2))

    in_maps = []
    for h in range(8):
        sl = slice(h * HC, (h + 1) * HC)
        wt = np.concatenate(
            [W_qkv[D + h * HC:D + (h + 1) * HC],        # k rows
             W_qkv[2 * D + h * HC:2 * D + (h + 1) * HC],  # v rows
             W_qkv[sl],                                   # q rows
             W_gate[sl]], axis=0                          # g rows
        ).T.astype(np.float16)                            # [D, 512]
        wt = np.ascontiguousarray(wt.reshape(KT, P, W4))
        m = {"xh": xh, "wt": wt, **consts}
        if with_bias:
            bq = np.asarray(b_qkv, dtype=np.float32)
            bg = np.asarray(b_gate, dtype=np.float32)
            m["bias"] = np.concatenate(
                [bq[D + h * HC:D + (h + 1) * HC],
                 bq[2 * D + h * HC:2 * D + (h + 1) * HC],
                 bq[sl], bg[sl]]
            )[None, :].astype(np.float16).copy()
        in_maps.append(m)
    return in_maps


def run(x, W_qkv, b_qkv, W_gate, b_gate, trace=False, **run_kwargs):
    with_bias = bool(np.any(np.asarray(b_qkv)) or np.any(np.asarray(b_gate)))
    nc = _get_nc(with_bias)
    in_maps = _prep_in_maps(x, W_qkv, b_qkv, W_gate, b_gate, with_bias)
    res = run_bass_kernel_spmd(nc, in_maps, list(range(8)), trace=trace, **run_kwargs)
    out = np.empty((B, S, D), dtype=np.float32)
    for h in range(8):
        # res[h]["out"]: [NBLK, P, B, HC] -> out[b, s, h*HC:(h+1)*HC]
        o = np.asarray(res.results[h]["out"]).transpose(2, 0, 1, 3)
        out[:, :, h * HC:(h + 1) * HC] = o.reshape(B, S, HC)
    return out, res


def kernel(x, W_qkv, b_qkv, W_gate, b_gate):
    out, _ = run(x, W_qkv, b_qkv, W_gate, b_gate)
    return out


# revision 21
# speedup vs baseline: 1.0129x; 1.0082x over previous
"""Trainium2 Bass kernel for nn_LinearLatentKernel_84834194031187.

Computes, for x:[B,S,D], W_qkv:[3D,D], W_gate:[D,D] (fp32):
    qkv = x @ W_qkv.T + b_qkv ; q,k,v = split(qkv)
    kv_state = cumsum(k*v, axis=seq)
    out = q * kv_state * sigmoid(x @ W_gate.T + b_gate)

Sharding: 8-way channel split. Core h handles channels [h*128,(h+1)*128) of
q, k, v and the gate for ALL four batches, producing out[:, :, h*128:...].
This keeps each core's weight slice at 1MB (vs 4MB for a batch x half-D
split), which matters because the startup ramp is HBM-bandwidth-bound:
block 0 cannot finish before its weights land.

x is host-pretransposed and pre-tiled into [NBLK, 128, B, KT, 128] fp16 so
each seq block's x^T tiles (all 4 batches) arrive via contiguous DMAs
(8KB/partition) and feed the PE stationary port directly -- no on-device
transposes.

Per seq block of 128 rows (partition dim = seq):
  - one PSUM bank per batch accumulates [k|v|q|g] x 128 channels over the
    8 contraction tiles (fp16 operands, fp32 PSUM, N=512 matmuls). The
    batch-outer matmul order staggers bank completion so each bank is
    drained (kv product, sigmoid, q*g on DVE/ACT) while later batches'
    matmuls still run: all four banks stay single-buffered with no stalls.
  - carry fold: kv_b[0,:] += carry_b; one upper-triangular fp16 matmul per
    batch then yields the block cumsum INCLUDING the carry, and its row 127
    IS the carry for the next block: a 1-lane scalar copy + SBUF-to-SBUF
    DMA (all 4 batches in one go) moves it from partition 127 to 0.
  - The PE queue is software-pipelined one block: cumsum matmuls for block
    i-1 are enqueued between batch groups of block i's projections.
  - out = (q * sigmoid(g)) * kv_state, one 256KB DMA per block.

A short burst of dummy "warmup" matmuls (no DMA dependencies) runs first so
the PE's HAM reaches its fast state during the initial DMA ramp.
"""

import numpy as np

import concourse.bacc as bacc
import concourse.tile as tile
import concourse.mybir as mybir
from concourse.bass_utils import run_bass_kernel_spmd

B, S, D = 4, 4096, 1024
HC = 128         # channels per core (D / 8 cores)
W4 = 4 * HC      # k|v|q|g channel block per core = 512
P = 128
NBLK = S // P    # 32 seq blocks
KT = D // P      # 8 contraction tiles

f32 = mybir.dt.float32
f16 = mybir.dt.float16

_NC_CACHE = {}


def _build(with_bias: bool):
    nc = bacc.Bacc("TRN2", target_bir_lowering=False)

    # x^T pre-tiled on host: xh[i, p, b, kt, j] = x[b, i*128+j, kt*128+p]
    xh_d = nc.dram_tensor("xh", [NBLK, P, B, KT, P], f16, kind="ExternalInput")
    # weight columns ordered [k | v | q | g], HC channels each
    wt_d = nc.dram_tensor("wt", [KT, P, W4], f16, kind="ExternalInput")
    tri_d = nc.dram_tensor("tri", [P, P], f16, kind="ExternalInput")
    if with_bias:
        onesrow_d = nc.dram_tensor("onesrow", [1, P], f16, kind="ExternalInput")
        bias_d = nc.dram_tensor("bias", [1, W4], f16, kind="ExternalInput")
    # out[i, j, b, c] = result[b, i*128+j, h*128+c]
    out_d = nc.dram_tensor("out", [NBLK, P, B, HC], f32, kind="ExternalOutput")

    with tile.TileContext(nc) as tc:
        with (
            tc.tile_pool(name="consts", bufs=1) as consts,
            tc.tile_pool(name="xtp", bufs=3) as xtp,
            tc.tile_pool(name="kp", bufs=2) as kp,
            tc.tile_pool(name="gp", bufs=2) as gp,
            tc.tile_pool(name="kvp", bufs=2) as kvp,
            tc.tile_pool(name="qgp", bufs=2) as qgp,
            tc.tile_pool(name="outp", bufs=3) as outp,
            tc.tile_pool(name="tmpp", bufs=2) as tmpp,
            tc.tile_pool(name="carryp", bufs=2) as carryp,
            tc.tile_pool(name="pmm", bufs=1, space="PSUM") as pmm,
            tc.tile_pool(name="pcs_pool", bufs=2, space="PSUM") as pcs_pool,
            tc.tile_pool(name="pwm", bufs=1, space="PSUM") as pwm,
        ):
            warm_a = consts.tile([P, P], f16, tag="warm_a")
            nc.vector.memset(warm_a[:], 0.0)
            warm_b = consts.tile([P, 512], f16, tag="warm_b")
            nc.vector.memset(warm_b[:], 0.0)
            pwarm = pwm.tile([P, 512], f32, tag="pwarm")

            def warm(n):
                for _ in range(n):
                    nc.tensor.matmul(pwarm[:], warm_a[:], warm_b[:],
                                     start=True, stop=True)

            warm(18)

            # block 0/1 inputs split across two trigger queues; weights on
            # gpsimd+scalar so everything rides different DMA rings
            xt0 = xtp.tile([P, B, KT, P], f16, tag="xt", name="xt0")
            nc.sync.dma_start(xt0[:, 0:2], xh_d[0, :, 0:2])
            nc.scalar.dma_start(xt0[:, 2:4], xh_d[0, :, 2:4])
            wt_sb = consts.tile([P, KT, W4], f16, tag="wt")
            for kt in range(KT):
                eng = nc.gpsimd if kt % 2 == 0 else nc.scalar
                eng.dma_start(wt_sb[:, kt, :], wt_d[kt])
            xt1 = xtp.tile([P, B, KT, P], f16, tag="xt", name="xt1")
            nc.sync.dma_start(xt1[:, 0:2], xh_d[1, :, 0:2])
            nc.scalar.dma_start(xt1[:, 2:4], xh_d[1, :, 2:4])

            tri_sb = consts.tile([P, P], f16, tag="tri")
            nc.sync.dma_start(tri_sb[:], tri_d[:])
            if with_bias:
                onesrow_sb = consts.tile([1, P], f16, tag="onesrow")
                nc.sync.dma_start(onesrow_sb[:], onesrow_d[:])
                bias_sb = consts.tile([1, W4], f16, tag="bias")
                nc.sync.dma_start(bias_sb[:], bias_d[:])

            xts = {0: xt0, 1: xt1}
            pending = None      # (kvs, qgs, i) awaiting cumsum+output

            def proj_batch(ps_b, xt, b):
                for kt in range(KT):
                    nc.tensor.matmul(
                        ps_b[:], xt[:, b, kt, :], wt_sb[:, kt, :],
                        start=(kt == 0),
                        stop=(kt == KT - 1 and not with_bias),
                    )
                if with_bias:
                    nc.tensor.matmul(ps_b[:], onesrow_sb[:], bias_sb[:],
                                     start=False, stop=True)

            def cumsum_mms(pend):
                # PE part of block j's cumsum, one tri matmul per batch into
                # quarter-bank slices; row 127 = next carry (all batches),
                # moved 127->0 via 1-lane copy + tiny SBUF DMA.
                kvs, qgs, j = pend
                pcs = pcs_pool.tile([P, B, HC], f32, tag="pcs")
                for b in range(B):
                    nc.tensor.matmul(pcs[:, b, :], tri_sb[:], kvs[b][:],
                                     start=True, stop=True)
                carry_new = None
                if j < NBLK - 1:
                    tmp = tmpp.tile([P, B, HC], f32, tag="tmp")
                    nc.scalar.activation(tmp[96:P], pcs[96:P],
                                         mybir.ActivationFunctionType.Copy)
                    carry_new = carryp.tile([1, B, HC], f32, tag="carry")
                    nc.gpsimd.dma_start(carry_new[0:1], tmp[P - 1:P])
                return pcs, carry_new

            def emit_out(pend, pcs):
                _, qgs, j = pend
                ob = outp.tile([P, B, HC], f32, tag="ob")
                for b in range(B):
                    nc.vector.tensor_mul(out=ob[:, b, :], in0=qgs[b][:],
                                         in1=pcs[:, b, :])
                nc.sync.dma_start(out_d[j], ob[:])

            for i in range(NBLK):
                if i + 2 < NBLK:
                    xt = xtp.tile([P, B, KT, P], f16, tag="xt")
                    nc.sync.dma_start(xt[:, 0:2], xh_d[i + 2, :, 0:2])
                    nc.scalar.dma_start(xt[:, 2:4], xh_d[i + 2, :, 2:4])
                    xts[i + 2] = xt
                xt = xts.pop(i)

                ps = [pmm.tile([P, W4], f32, tag=f"ps{b}", name=f"ps{b}")
                      for b in range(B)]
                kvs, qgs = [], []

                proj_batch(ps[0], xt, 0)
                proj_batch(ps[1], xt, 1)

                # block i-1's cumsum matmuls, mid-block on the PE queue
                pcs_prev = None
                if pending is not None:
                    pcs_prev, carry_prev = cumsum_mms(pending)

                proj_batch(ps[2], xt, 2)
                proj_batch(ps[3], xt, 3)

                for b in range(B):
                    k_sb = kp.tile([P, HC], f32, tag=f"k{b}")
                    nc.scalar.activation(k_sb[:], ps[b][:, 0:HC],
                                         mybir.ActivationFunctionType.Copy)
                    kv = kvp.tile([P, HC], f16, tag=f"kv{b}")
                    nc.vector.tensor_mul(out=kv[:], in0=k_sb[:],
                                         in1=ps[b][:, HC:2 * HC])
                    kvs.append(kv)
                    if b == 0 and pending is not None:
                        emit_out(pending, pcs_prev)
                    g_sb = gp.tile([P, HC], f32, tag=f"g{b}")
                    nc.scalar.activation(g_sb[:], ps[b][:, 3 * HC:4 * HC],
                                         mybir.ActivationFunctionType.Sigmoid)
                    qg = qgp.tile([P, HC], f32, tag=f"qg{b}")
                    nc.vector.tensor_mul(out=qg[:], in0=g_sb[:],
                                         in1=ps[b][:, 2 * HC:3 * HC])
                    qgs.append(qg)

                if i > 0:
                    for b in range(B):
                        # carry fold: kv_b[0,:] += carry (cumsum row 127)
                        nc.vector.tensor_add(
                            out=kvs[b][0:1, :], in0=kvs[b][0:1, :],
                            in1=carry_prev[0:1, b, :])
                pending = (kvs, qgs, i)

            pcs_last, _ = cumsum_mms(pending)
            emit_out(pending, pcs_last)

    nc.compile()
    return nc


def _get_nc(with_bias: bool):
    if with_bias not in _NC_CACHE:
        _NC_CACHE[with_bias] = _build(with_bias)
    return _NC_CACHE[with_bias]


def _prep_in_maps(x, W_qkv, b_qkv, W_gate, b_gate, with_bias):
    x = np.asarray(x, dtype=np.float32).astype(np.float16)
    W_qkv = np.asarray(W_qkv, dtype=np.float32)
    W_gate = np.asarray(W_gate, dtype=np.float32)

    consts = {
        "tri": np.triu(np.ones((P, P), dtype=np.float16)),
    }
    if with_bias:
        consts["onesrow"] = np.ones((1, P), dtype=np.float16)

    # xh[i, p, b, kt, j] = x[b, i*128+j, kt*128+p]  (shared by all cores)
    xh = np.ascontiguousarray(
        x.reshape(B, NBLK, P, KT, P).transpose(1, 4, 0, 3, 2))

    in_maps = []
    for h in range(8):
        sl = slice(h * HC, (h + 1) * HC)
        wt = np.concatenate(
            [W_qkv[D + h * HC:D + (h + 1) * HC],        # k rows
             W_qkv[2 * D + h * HC:2 * D + (h + 1) * HC],  # v rows
             W_qkv[sl],                                   # q rows
             W_gate[sl]], axis=0                          # g rows
        ).T.astype(np.float16)                            # [D, 512]
        wt = np.ascontiguousarray(wt.reshape(KT, P, W4))
        m = {"xh": xh, "wt": wt, **consts}
        if with_bias:
            bq = np.asarray(b_qkv, dtype=np.float32)
            bg = np.asarray(b_gate, dtype=np.float32)
            m["bias"] = np.concatenate(
                [bq[D + h * HC:D + (h + 1) * HC],
                 bq[2 * D + h * HC:2 * D + (h + 1) * HC],
                 bq[sl], bg[sl]]
            )[None, :].astype(np.float16).copy()
        in_maps.append(m)
    return in_maps


def run(x, W_qkv, b_qkv, W_gate, b_gate, trace=False, **run_kwargs):
    with_bias = bool(np.any(np.asarray(b_qkv)) or np.any(np.asarray(b_gate)))
    nc = _get_nc(with_bias)
    in_maps = _prep_in_maps(x, W_qkv, b_qkv, W_gate, b_gate, with_bias)
    res = run_bass_kernel_spmd(nc, in_maps, list(range(8)), trace=trace, **run_kwargs)
    out = np.empty((B, S, D), dtype=np.float32)
    for h in range(8):
        # res[h]["out"]: [NBLK, P, B, HC] -> out[b, s, h*HC:(h+1)*HC]
        o = np.asarray(res.results[h]["out"]).transpose(2, 0, 1, 3)
        out[:, :, h * HC:(h + 1) * HC] = o.reshape(B, S, HC)
    return out, res


def kernel(x, W_qkv, b_qkv, W_gate, b_gate):
    out, _ = run(x, W_qkv, b_qkv, W_gate, b_gate)
    return out


# revision 25
# speedup vs baseline: 1.0131x; 1.0002x over previous
"""Trainium2 Bass kernel for nn_LinearLatentKernel_84834194031187.

Computes, for x:[B,S,D], W_qkv:[3D,D], W_gate:[D,D] (fp32):
    qkv = x @ W_qkv.T + b_qkv ; q,k,v = split(qkv)
    kv_state = cumsum(k*v, axis=seq)
    out = q * kv_state * sigmoid(x @ W_gate.T + b_gate)

Sharding: 8-way channel split. Core h handles channels [h*128,(h+1)*128) of
q, k, v and the gate for ALL four batches, producing out[:, :, h*128:...].
This keeps each core's weight slice at 1MB (vs 4MB for a batch x half-D
split), which matters because the startup ramp is HBM-bandwidth-bound:
block 0 cannot finish before its weights land.

x is host-pretransposed and pre-tiled into [NBLK, 128, B, KT, 128] fp16 so
each seq block's x^T tiles (all 4 batches) arrive via contiguous DMAs
(8KB/partition) and feed the PE stationary port directly -- no on-device
transposes.

Per seq block of 128 rows (partition dim = seq):
  - one PSUM bank per batch accumulates [k|v|q|g] x 128 channels over the
    8 contraction tiles (fp16 operands, fp32 PSUM, N=512 matmuls). The
    batch-outer matmul order staggers bank completion so each bank is
    drained (kv product, sigmoid, q*g on DVE/ACT) while later batches'
    matmuls still run: all four banks stay single-buffered with no stalls.
  - carry fold: kv_b[0,:] += carry_b; one upper-triangular fp16 matmul per
    batch then yields the block cumsum INCLUDING the carry, and its row 127
    IS the carry for the next block: a 1-lane scalar copy + SBUF-to-SBUF
    DMA (all 4 batches in one go) moves it from partition 127 to 0.
  - The PE queue is software-pipelined one block: cumsum matmuls for block
    i-1 are enqueued between batch groups of block i's projections.
  - out = (q * sigmoid(g)) * kv_state, one 256KB DMA per block.

A short burst of dummy "warmup" matmuls (no DMA dependencies) runs first so
the PE's HAM reaches its fast state during the initial DMA ramp.
"""

import numpy as np

import concourse.bacc as bacc
import concourse.tile as tile
import concourse.mybir as mybir
from concourse.bass_utils import run_bass_kernel_spmd

B, S, D = 4, 4096, 1024
HC = 128         # channels per core (D / 8 cores)
W4 = 4 * HC      # k|v|q|g channel block per core = 512
P = 128
NBLK = S // P    # 32 seq blocks
KT = D // P      # 8 contraction tiles

f32 = mybir.dt.float32
f16 = mybir.dt.float16

_NC_CACHE = {}


def _build(with_bias: bool):
    nc = bacc.Bacc("TRN2", target_bir_lowering=False)

    # x^T pre-tiled on host: xh[i, p, b, kt, j] = x[b, i*128+j, kt*128+p]
    xh_d = nc.dram_tensor("xh", [NBLK, P, B, KT, P], f16, kind="ExternalInput")
    # weight columns ordered [k | v | q | g], HC channels each
    wt_d = nc.dram_tensor("wt", [KT, P, W4], f16, kind="ExternalInput")
    tri_d = nc.dram_tensor("tri", [P, P], f16, kind="ExternalInput")
    if with_bias:
        onesrow_d = nc.dram_tensor("onesrow", [1, P], f16, kind="ExternalInput")
        bias_d = nc.dram_tensor("bias", [1, W4], f16, kind="ExternalInput")
    # out[i, j, b, c] = result[b, i*128+j, h*128+c]
    out_d = nc.dram_tensor("out", [NBLK, P, B, HC], f32, kind="ExternalOutput")

    with tile.TileContext(nc) as tc:
        with (
            tc.tile_pool(name="consts", bufs=1) as consts,
            tc.tile_pool(name="xtp", bufs=3) as xtp,
            tc.tile_pool(name="kp", bufs=2) as kp,
            tc.tile_pool(name="gp", bufs=2) as gp,
            tc.tile_pool(name="kvp", bufs=2) as kvp,
            tc.tile_pool(name="qgp", bufs=2) as qgp,
            tc.tile_pool(name="outp", bufs=3) as outp,
            tc.tile_pool(name="tmpp", bufs=2) as tmpp,
            tc.tile_pool(name="carryp", bufs=2) as carryp,
            tc.tile_pool(name="pmm", bufs=1, space="PSUM") as pmm,
            tc.tile_pool(name="pcs_pool", bufs=2, space="PSUM") as pcs_pool,
            tc.tile_pool(name="pwm", bufs=1, space="PSUM") as pwm,
        ):
            warm_a = consts.tile([P, P], f16, tag="warm_a")
            nc.vector.memset(warm_a[:], 0.0)
            warm_b = consts.tile([P, 512], f16, tag="warm_b")
            nc.vector.memset(warm_b[:], 0.0)
            pwarm = pwm.tile([P, 512], f32, tag="pwarm")

            def warm(n):
                for _ in range(n):
                    nc.tensor.matmul(pwarm[:], warm_a[:], warm_b[:],
                                     start=True, stop=True)

            warm(18)

            # block 0/1 inputs split across two trigger queues; weights on
            # gpsimd+scalar so everything rides different DMA rings
            xt0 = xtp.tile([P, B, KT, P], f16, tag="xt", name="xt0")
            nc.sync.dma_start(xt0[:, 0:2], xh_d[0, :, 0:2])
            nc.scalar.dma_start(xt0[:, 2:4], xh_d[0, :, 2:4])
            wt_sb = consts.tile([P, KT, W4], f16, tag="wt")
            for kt in range(KT):
                eng = nc.gpsimd if kt % 2 == 0 else nc.scalar
                eng.dma_start(wt_sb[:, kt, :], wt_d[kt])
            xt1 = xtp.tile([P, B, KT, P], f16, tag="xt", name="xt1")
            nc.sync.dma_start(xt1[:, 0:2], xh_d[1, :, 0:2])
            nc.scalar.dma_start(xt1[:, 2:4], xh_d[1, :, 2:4])

            tri_sb = consts.tile([P, P], f16, tag="tri")
            nc.sync.dma_start(tri_sb[:], tri_d[:])
            if with_bias:
                onesrow_sb = consts.tile([1, P], f16, tag="onesrow")
                nc.sync.dma_start(onesrow_sb[:], onesrow_d[:])
                bias_sb = consts.tile([1, W4], f16, tag="bias")
                nc.sync.dma_start(bias_sb[:], bias_d[:])

            xts = {0: xt0, 1: xt1}
            pending = None      # (kvs, qgs, i) awaiting cumsum+output

            def proj_batch(ps_b, xt, b):
                for kt in range(KT):
                    nc.tensor.matmul(
                        ps_b[:], xt[:, b, kt, :], wt_sb[:, kt, :],
                        start=(kt == 0),
                        stop=(kt == KT - 1 and not with_bias),
                    )
                if with_bias:
                    nc.tensor.matmul(ps_b[:], onesrow_sb[:], bias_sb[:],
                                     start=False, stop=True)

            def cumsum_mms(pend):
                # PE part of block j's cumsum, one tri matmul per batch into
                # quarter-bank slices; row 127 = next carry (all batches),
                # moved 127->0 via 1-lane copy + tiny SBUF DMA.
                kvs, qgs, j = pend
                pcs = pcs_pool.tile([P, B, HC], f32, tag="pcs")
                for b in range(B):
                    nc.tensor.matmul(pcs[:, b, :], tri_sb[:], kvs[b][:],
                                     start=True, stop=True)
                carry_new = None
                if j < NBLK - 1:
                    tmp = tmpp.tile([P, B, HC], f32, tag="tmp")
                    nc.scalar.activation(tmp[96:P], pcs[96:P],
                                         mybir.ActivationFunctionType.Copy)
                    carry_new = carryp.tile([1, B, HC], f32, tag="carry")
                    nc.gpsimd.dma_start(carry_new[0:1], tmp[P - 1:P])
                return pcs, carry_new

            def emit_out(pend, pcs):
                _, qgs, j = pend
                ob = outp.tile([P, B, HC], f32, tag="ob")
                for b in range(B):
                    nc.vector.tensor_mul(out=ob[:, b, :], in0=qgs[b][:],
                                         in1=pcs[:, b, :])
                nc.sync.dma_start(out_d[j], ob[:])

            for i in range(NBLK):
                if i + 2 < NBLK:
                    xt = xtp.tile([P, B, KT, P], f16, tag="xt")
                    nc.sync.dma_start(xt[:, 0:2], xh_d[i + 2, :, 0:2])
                    nc.scalar.dma_start(xt[:, 2:4], xh_d[i + 2, :, 2:4])
                    xts[i + 2] = xt
                xt = xts.pop(i)

                ps = [pmm.tile([P, W4], f32, tag=f"ps{b}", name=f"ps{b}")
                      for b in range(B)]
                kvs, qgs = [], []

                proj_batch(ps[0], xt, 0)
                proj_batch(ps[1], xt, 1)

                # block i-1's cumsum matmuls, mid-block on the PE queue
                pcs_prev = None
                if pending is not None:
                    pcs_prev, carry_prev = cumsum_mms(pending)

                proj_batch(ps[2], xt, 2)
                proj_batch(ps[3], xt, 3)

                for b in range(B):
                    k_sb = kp.tile([P, HC], f32, tag=f"k{b}")
                    nc.scalar.activation(k_sb[:], ps[b][:, 0:HC],
                                         mybir.ActivationFunctionType.Copy)
                    kv = kvp.tile([P, HC], f16, tag=f"kv{b}")
                    nc.vector.tensor_mul(out=kv[:], in0=k_sb[:],
                                         in1=ps[b][:, HC:2 * HC])
                    kvs.append(kv)
                    if b == 0 and pending is not None:
                        emit_out(pending, pcs_prev)
                    g_sb = gp.tile([P, HC], f32, tag=f"g{b}")
                    nc.scalar.activation(g_sb[:], ps[b][:, 3 * HC:4 * HC],
                                         mybir.ActivationFunctionType.Sigmoid)
                    qg = qgp.tile([P, HC], f32, tag=f"qg{b}")
                    nc.vector.tensor_mul(out=qg[:], in0=g_sb[:],
                                         in1=ps[b][:, 2 * HC:3 * HC])
                    qgs.append(qg)
                    if i > 0:
                        # carry fold: kv_b[0,:] += carry (cumsum row 127)
                        nc.vector.tensor_add(
                            out=kv[0:1, :], in0=kv[0:1, :],
                            in1=carry_prev[0:1, b, :])

                pending = (kvs, qgs, i)

            pcs_last, _ = cumsum_mms(pending)
            emit_out(pending, pcs_last)

    nc.compile()
    return nc


def _get_nc(with_bias: bool):
    if with_bias not in _NC_CACHE:
        _NC_CACHE[with_bias] = _build(with_bias)
    return _NC_CACHE[with_bias]


def _prep_in_maps(x, W_qkv, b_qkv, W_gate, b_gate, with_bias):
    x = np.asarray(x, dtype=np.float32).astype(np.float16)
    W_qkv = np.asarray(W_qkv, dtype=np.float32)
    W_gate = np.asarray(W_gate, dtype=np.float32)

    consts = {
        "tri": np.triu(np.ones((P, P), dtype=np.float16)),
    }
    if with_bias:
        consts["onesrow"] = np.ones((1, P), dtype=np.float16)

    # xh[i, p, b, kt, j] = x[b, i*128+j, kt*128+p]  (shared by all cores)
    xh = np.ascontiguousarray(
        x.reshape(B, NBLK, P, KT, P).transpose(1, 4, 0, 3, 2))

    in_maps = []
    for h in range(8):
        sl = slice(h * HC, (h + 1) * HC)
        wt = np.concatenate(
            [W_qkv[D + h * HC:D + (h + 1) * HC],        # k rows
             W_qkv[2 * D + h * HC:2 * D + (h + 1) * HC],  # v rows
             W_qkv[sl],                                   # q rows
             W_gate[sl]], axis=0                          # g rows
        ).T.astype(np.float16)                            # [D, 512]
        wt = np.ascontiguousarray(wt.reshape(KT, P, W4))
        m = {"xh": xh, "wt": wt, **consts}
        if with_bias:
            bq = np.asarray(b_qkv, dtype=np.float32)
            bg = np.asarray(b_gate, dtype=np.float32)
            m["bias"] = np.concatenate(
                [bq[D + h * HC:D + (h + 1) * HC],
                 bq[2 * D + h * HC:2 * D + (h + 1) * HC],
                 bq[sl], bg[sl]]
            )[None, :].astype(np.float16).copy()
        in_maps.append(m)
    return in_maps


def run(x, W_qkv, b_qkv, W_gate, b_gate, trace=False, **run_kwargs):
    with_bias = bool(np.any(np.asarray(b_qkv)) or np.any(np.asarray(b_gate)))
    nc = _get_nc(with_bias)
    in_maps = _prep_in_maps(x, W_qkv, b_qkv, W_gate, b_gate, with_bias)
    res = run_bass_kernel_spmd(nc, in_maps, list(range(8)), trace=trace, **run_kwargs)
    out = np.empty((B, S, D), dtype=np.float32)
    for h in range(8):
        # res[h]["out"]: [NBLK, P, B, HC] -> out[b, s, h*HC:(h+1)*HC]
        o = np.asarray(res.results[h]["out"]).transpose(2, 0, 1, 3)
        out[:, :, h * HC:(h + 1) * HC] = o.reshape(B, S, HC)
    return out, res


def kernel(x, W_qkv, b_qkv, W_gate, b_gate):
    out, _ = run(x, W_qkv, b_qkv, W_gate, b_gate)
    return out
